# revision 2
# baseline (speedup 1.0000x reference)
"""MoE routed decoder kernel for 8 Trainium2 NeuronCores.

Strategy:
  - Host: compute per-row expert routes (int truncation tests on the last 3
    features), group rows by expert into 128-row blocks (padded by repeating a
    real row), build x^T in sorted order.
  - Device (SPMD, 8 cores): every core computes layer 1 (relu(x @ W1[e]))
    for all sorted rows in bf16 (f32 accumulate), then layer 2 for its own
    1024-wide slice of the 8192 output columns, then the complex-pair L2
    normalization (a free-axis reduction over 256-column groups).
  - Host: stitch the 8 column slices, undo the sort, reshape to (B, 32, 256).

All matmul inputs are cast to bf16 on host (halves weight DMA; PE runs bf16 at
4x the fp32 rate); accumulation stays fp32 in PSUM, normalization in fp32.
"""

import os
import sys
import types

import numpy as np
import ml_dtypes

import concourse.bass as bass
import concourse.mybir as mybir
import concourse.tile as tile
from concourse import bacc
import concourse.bass_utils as bass_utils
from concourse.bass_utils import run_bass_kernel_spmd

B, D, H, O, E, P = 1024, 512, 2048, 8192, 5, 128
NCORES = 8
OSL = O // NCORES  # output columns per core
BF16 = mybir.dt.bfloat16
F32 = mybir.dt.float32
AF = mybir.ActivationFunctionType

# Filled by the last kernel() call when tracing is enabled (BASSMOE_TRACE=1).
LAST_EXEC_NS = None
LAST_TRACE = None


def _install_ntff_hook():
    """Best-effort NTFF profile hook for exec-time measurement under axon."""
    try:
        import trn_agent_boot.trn_boot as tb

        hook = tb._ntff_profile_via_ctypes("/opt/axon/libaxon_pjrt.so")
        mod = types.ModuleType("antenv.axon_hooks")
        mod.get_axon_ntff_profile_hook = lambda: hook
        import antenv

        antenv.axon_hooks = mod
        sys.modules["antenv.axon_hooks"] = mod
        bass_utils.upload_artifacts = lambda tmpdir: tmpdir  # no S3 in container
        return True
    except Exception:
        return False


def _route(x):
    c1 = x[:, -1].astype(np.int32) == 0
    c2 = x[:, -2].astype(np.int32) == 0
    c3 = x[:, -3].astype(np.int32) == 0
    r_if = np.where(c2, 0, np.where(c3, 3, 4))
    r_else = np.where(c2, 1, 2)
    return np.where(c1, r_if, r_else).astype(np.int64)


def _plan(route):
    """Group rows by expert into 128-row blocks.

    Returns (pad_idx, valid, segs, block_expert):
      pad_idx[i]   original row feeding sorted position i (len NP, mult of 128)
      valid[i]     True where position i carries a real (non-padding) row
      segs         [(expert, col_start, col_len)] with 128-aligned extents
      block_expert expert id for each 128-row block
    """
    pad_idx, valid, segs, block_expert = [], [], [], []
    for e in range(E):
        idx = np.nonzero(route == e)[0]
        n = len(idx)
        if n == 0:
            continue
        nb = -(-n // P)
        c0 = len(pad_idx)
        pad_idx.extend(idx.tolist())
        pad_idx.extend([int(idx[0])] * (nb * P - n))
        valid.extend([True] * n + [False] * (nb * P - n))
        segs.append((e, c0, nb * P))
        block_expert.extend([e] * nb)
    return (
        np.array(pad_idx, dtype=np.int64),
        np.array(valid, dtype=bool),
        segs,
        block_expert,
    )


def _build_program(NP, segs, block_expert, b1_nz, b2_nz):
    NBLK = NP // P
    KC1 = D // P  # 4
    HM = H // P  # 16
    KC2 = H // P  # 16

    nc = bacc.Bacc("TRN2", target_bir_lowering=False, debug=False,
                   num_devices=NCORES)
    XT = nc.dram_tensor("xt", [D, NP], BF16, kind="ExternalInput").ap()
    W1T = nc.dram_tensor("w1", [E, D, H], BF16, kind="ExternalInput").ap()
    B1T = nc.dram_tensor("b1", [E, H], F32, kind="ExternalInput").ap()
    W2T = nc.dram_tensor("w2", [E, H, OSL], BF16, kind="ExternalInput").ap()
    B2T = nc.dram_tensor("b2", [E, OSL], F32, kind="ExternalInput").ap()
    OUT = nc.dram_tensor("out", [NP, OSL], F32, kind="ExternalOutput").ap()

    with tile.TileContext(nc) as tc:
        with (
            tc.tile_pool(name="singles", bufs=1) as singles,
            tc.tile_pool(name="w1p", bufs=2) as w1p,
            tc.tile_pool(name="w2p", bufs=2) as w2p,
            tc.tile_pool(name="ps1", bufs=3, space="PSUM") as ps1,
            tc.tile_pool(name="ps2", bufs=4, space="PSUM") as ps2,
            tc.tile_pool(name="sqp", bufs=2) as sqp,
            tc.tile_pool(name="outp", bufs=3) as outp,
            tc.tile_pool(name="nrmp", bufs=4) as nrmp,
        ):
            xt_sb = singles.tile([P, KC1, NP], BF16)
            nc.sync.dma_start(xt_sb[:], XT.rearrange("(kc p) n -> p kc n", p=P))

            h_sb = singles.tile([P, HM, NP], BF16)

            b1_sb = None
            if b1_nz:
                b1_sb = singles.tile([P, E, HM], F32)
                nc.sync.dma_start(
                    b1_sb[:], B1T.rearrange("e (hm p) -> p e hm", p=P)
                )
            b2_sb = None
            if b2_nz:
                b2_sb = singles.tile([P, E, OSL], F32)
                bcast = bass.AP(
                    tensor=B2T.tensor,
                    offset=B2T.offset,
                    ap=[[0, P], *B2T.ap],
                )
                nc.sync.dma_start(b2_sb[:], bcast)

            # ---- layer 1: h^T = relu(W1[e]^T x^T) per expert segment ----
            for e, c0, clen in segs:
                w1t = w1p.tile([P, KC1, H], BF16, tag="w1")
                nc.sync.dma_start(
                    w1t[:], W1T[e].rearrange("(kc p) m -> p kc m", p=P)
                )
                for hm in range(HM):
                    for nch in range(0, clen, 512):
                        nn = min(512, clen - nch)
                        ps = ps1.tile([P, 512], F32, tag="ps1")
                        for kc in range(KC1):
                            nc.tensor.matmul(
                                ps[:, :nn],
                                w1t[:, kc, hm * P:(hm + 1) * P],
                                xt_sb[:, kc, c0 + nch:c0 + nch + nn],
                                start=(kc == 0),
                                stop=(kc == KC1 - 1),
                            )
                        bias = b1_sb[:, e, hm:hm + 1] if b1_nz else 0.0
                        nc.scalar.activation(
                            h_sb[:, hm, c0 + nch:c0 + nch + nn],
                            ps[:, :nn],
                            AF.Relu,
                            bias=bias,
                        )

            # ---- layer 2 + complex-pair normalization, per 128-row block ----
            cur_e = None
            w2t = None
            for mb in range(NBLK):
                e = block_expert[mb]
                if e != cur_e:
                    w2t = w2p.tile([P, KC2, OSL], BF16, tag="w2")
                    nc.sync.dma_start(
                        w2t[:], W2T[e].rearrange("(kc p) n -> p kc n", p=P)
                    )
                    cur_e = e
                ps_a = ps2.tile([P, 512], F32, tag="ps2")
                ps_b = ps2.tile([P, 512], F32, tag="ps2")
                for kc in range(KC2):
                    lhsT = h_sb[:, kc, mb * P:(mb + 1) * P]
                    nc.tensor.matmul(ps_a, lhsT, w2t[:, kc, 0:512],
                                     start=(kc == 0), stop=(kc == KC2 - 1))
                    nc.tensor.matmul(ps_b, lhsT, w2t[:, kc, 512:OSL],
                                     start=(kc == 0), stop=(kc == KC2 - 1))

                halves = [ps_a, ps_b]
                if b2_nz:
                    osb = outp.tile([P, OSL], F32, tag="osb")
                    for i, ph in enumerate(halves):
                        nc.vector.tensor_add(
                            osb[:, i * 512:(i + 1) * 512], ph[:],
                            b2_sb[:, e, i * 512:(i + 1) * 512],
                        )
                    halves = [osb[:, 0:512], osb[:, 512:OSL]]

                sq = sqp.tile([P, OSL], F32, tag="sq")
                for i, ph in enumerate(halves):
                    nc.scalar.activation(sq[:, i * 512:(i + 1) * 512], ph[:],
                                         AF.Square)
                nrm = nrmp.tile([P, OSL // 256], F32, tag="nrm")
                nc.vector.reduce_sum(
                    nrm[:],
                    sq.rearrange("p (w d) -> p w d", d=256),
                    axis=mybir.AxisListType.X,
                )
                nc.scalar.sqrt(nrm[:], nrm[:])
                rn = nrmp.tile([P, OSL // 256], F32, tag="rn")
                nc.vector.reciprocal(rn[:], nrm[:])

                out_sb = outp.tile([P, OSL], F32, tag="onorm")
                for w in range(OSL // 256):
                    src = halves[(w * 256) // 512]
                    off = (w * 256) % 512
                    nc.vector.tensor_scalar_mul(
                        out_sb[:, w * 256:(w + 1) * 256],
                        src[:, off:off + 256],
                        rn[:, w:w + 1],
                    )
                nc.sync.dma_start(OUT[mb * P:(mb + 1) * P, :], out_sb[:])

    nc.compile()
    return nc


def kernel(x, W1, b1, W2, b2):
    x = np.asarray(x, dtype=np.float32)
    W1 = np.asarray(W1, dtype=np.float32)
    b1 = np.asarray(b1, dtype=np.float32)
    W2 = np.asarray(W2, dtype=np.float32)
    b2 = np.asarray(b2, dtype=np.float32)

    route = _route(x)
    pad_idx, valid, segs, block_expert = _plan(route)
    NP = len(pad_idx)

    xt = np.ascontiguousarray(x[pad_idx].T.astype(ml_dtypes.bfloat16))
    w1b = W1.astype(ml_dtypes.bfloat16)
    w2b = W2.astype(ml_dtypes.bfloat16)

    b1_nz = bool(np.any(b1))
    b2_nz = bool(np.any(b2))

    nc = _build_program(NP, segs, block_expert, b1_nz, b2_nz)

    in_maps = []
    for c in range(NCORES):
        sl = slice(c * OSL, (c + 1) * OSL)
        in_maps.append({
            "xt": xt,
            "w1": w1b,
            "b1": b1,
            "w2": np.ascontiguousarray(w2b[:, :, sl]),
            "b2": np.ascontiguousarray(b2[:, sl]),
        })

    trace = os.environ.get("BASSMOE_TRACE", "") == "1"
    if trace:
        trace = _install_ntff_hook()

    res = run_bass_kernel_spmd(
        nc, in_maps, core_ids=list(range(NCORES)), trace=trace,
        tmpdir=os.environ.get("BASSMOE_TRACE_DIR") or None,
    )
    global LAST_EXEC_NS, LAST_TRACE
    LAST_EXEC_NS = res.exec_time_ns
    LAST_TRACE = res.instructions_and_trace[1] if res.instructions_and_trace else None

    out_sorted = np.concatenate(
        [res.results[c]["out"] for c in range(NCORES)], axis=1
    )
    out = np.empty((B, O), dtype=np.float32)
    out[pad_idx[valid]] = out_sorted[valid]
    return out.reshape(B, 32, 256)


# revision 4
# speedup vs baseline: 1.0490x; 1.0490x over previous
"""MoE routed decoder kernel for 8 Trainium2 NeuronCores.

Strategy:
  - Host: compute per-row expert routes (int truncation tests on the last 3
    features), group rows by expert into 128-row blocks (padded by repeating a
    real row), build x^T in sorted order.
  - Device (SPMD, 8 cores): every core computes layer 1 (relu(x @ W1[e]))
    for all sorted rows in bf16 (f32 accumulate), then layer 2 for its own
    1024-wide slice of the 8192 output columns, then the complex-pair L2
    normalization (a free-axis reduction over 256-column groups).
  - Host: stitch the 8 column slices, undo the sort, reshape to (B, 32, 256).

All matmul inputs are cast to bf16 on host (halves weight DMA; PE runs bf16 at
4x the fp32 rate); accumulation stays fp32 in PSUM, normalization in fp32.
"""

import os
import sys
import types

import numpy as np
import ml_dtypes

import concourse.bass as bass
import concourse.mybir as mybir
import concourse.tile as tile
from concourse import bacc
import concourse.bass_utils as bass_utils
from concourse.bass_utils import run_bass_kernel_spmd

B, D, H, O, E, P = 1024, 512, 2048, 8192, 5, 128
NCORES = 8
OSL = O // NCORES  # output columns per core
BF16 = mybir.dt.bfloat16
F32 = mybir.dt.float32
AF = mybir.ActivationFunctionType

# Filled by the last kernel() call when tracing is enabled (BASSMOE_TRACE=1).
LAST_EXEC_NS = None
LAST_TRACE = None


def _install_ntff_hook():
    """Best-effort NTFF profile hook for exec-time measurement under axon."""
    try:
        import trn_agent_boot.trn_boot as tb

        hook = tb._ntff_profile_via_ctypes("/opt/axon/libaxon_pjrt.so")
        mod = types.ModuleType("antenv.axon_hooks")
        mod.get_axon_ntff_profile_hook = lambda: hook
        import antenv

        antenv.axon_hooks = mod
        sys.modules["antenv.axon_hooks"] = mod
        bass_utils.upload_artifacts = lambda tmpdir: tmpdir  # no S3 in container
        return True
    except Exception:
        return False


def _route(x):
    c1 = x[:, -1].astype(np.int32) == 0
    c2 = x[:, -2].astype(np.int32) == 0
    c3 = x[:, -3].astype(np.int32) == 0
    r_if = np.where(c2, 0, np.where(c3, 3, 4))
    r_else = np.where(c2, 1, 2)
    return np.where(c1, r_if, r_else).astype(np.int64)


def _plan(route):
    """Group rows by expert into 128-row blocks.

    Returns (pad_idx, valid, segs, block_expert):
      pad_idx[i]   original row feeding sorted position i (len NP, mult of 128)
      valid[i]     True where position i carries a real (non-padding) row
      segs         [(expert, col_start, col_len)] with 128-aligned extents
      block_expert expert id for each 128-row block
    """
    pad_idx, valid, segs, block_expert = [], [], [], []
    for e in range(E):
        idx = np.nonzero(route == e)[0]
        n = len(idx)
        if n == 0:
            continue
        nb = -(-n // P)
        c0 = len(pad_idx)
        pad_idx.extend(idx.tolist())
        pad_idx.extend([int(idx[0])] * (nb * P - n))
        valid.extend([True] * n + [False] * (nb * P - n))
        segs.append((e, c0, nb * P))
        block_expert.extend([e] * nb)
    return (
        np.array(pad_idx, dtype=np.int64),
        np.array(valid, dtype=bool),
        segs,
        block_expert,
    )


def _build_program(NP, segs, block_expert, b1_nz, b2_nz):
    NBLK = NP // P
    KC1 = D // P  # 4
    HM = H // P  # 16
    KC2 = H // P  # 16

    nc = bacc.Bacc("TRN2", target_bir_lowering=False, debug=False,
                   num_devices=NCORES)
    XT = nc.dram_tensor("xt", [D, NP], BF16, kind="ExternalInput").ap()
    W1T = nc.dram_tensor("w1", [E, D, H], BF16, kind="ExternalInput").ap()
    B1T = nc.dram_tensor("b1", [E, H], F32, kind="ExternalInput").ap()
    W2T = nc.dram_tensor("w2", [E, H, OSL], BF16, kind="ExternalInput").ap()
    B2T = nc.dram_tensor("b2", [E, OSL], F32, kind="ExternalInput").ap()
    OUT = nc.dram_tensor("out", [NP, OSL], F32, kind="ExternalOutput").ap()

    with tile.TileContext(nc) as tc:
        with (
            tc.tile_pool(name="singles", bufs=1) as singles,
            tc.tile_pool(name="w1p", bufs=2) as w1p,
            tc.tile_pool(name="w2p", bufs=3) as w2p,
            tc.tile_pool(name="ps1", bufs=3, space="PSUM") as ps1,
            tc.tile_pool(name="ps2", bufs=4, space="PSUM") as ps2,
            tc.tile_pool(name="sqp", bufs=2) as sqp,
            tc.tile_pool(name="outp", bufs=3) as outp,
            tc.tile_pool(name="nrmp", bufs=4) as nrmp,
        ):
            # per-K-chunk tiles so the first matmuls only wait on the first
            # 0.33 MB of DMA, not the whole 1.3 MB input
            xt_sb = []
            for kc in range(KC1):
                t = singles.tile([P, NP], BF16, tag=f"xt_{kc}")
                nc.sync.dma_start(t[:], XT[kc * P:(kc + 1) * P, :])
                xt_sb.append(t)

            h_sb = singles.tile([P, HM, NP], BF16)

            b1_sb = None
            if b1_nz:
                b1_sb = singles.tile([P, E, HM], F32)
                nc.sync.dma_start(
                    b1_sb[:], B1T.rearrange("e (hm p) -> p e hm", p=P)
                )
            b2_sb = None
            if b2_nz:
                b2_sb = singles.tile([P, E, OSL], F32)
                bcast = bass.AP(
                    tensor=B2T.tensor,
                    offset=B2T.offset,
                    ap=[[0, P], *B2T.ap],
                )
                nc.sync.dma_start(b2_sb[:], bcast)

            # ---- layer 1: h^T = relu(W1[e]^T x^T) per expert segment ----
            for e, c0, clen in segs:
                w1t = []
                for kc in range(KC1):
                    t = w1p.tile([P, H], BF16, tag=f"w1_{kc}")
                    nc.sync.dma_start(t[:], W1T[e, kc * P:(kc + 1) * P, :])
                    w1t.append(t)
                for hm in range(HM):
                    for nch in range(0, clen, 512):
                        nn = min(512, clen - nch)
                        ps = ps1.tile([P, 512], F32, tag="ps1")
                        for kc in range(KC1):
                            nc.tensor.matmul(
                                ps[:, :nn],
                                w1t[kc][:, hm * P:(hm + 1) * P],
                                xt_sb[kc][:, c0 + nch:c0 + nch + nn],
                                start=(kc == 0),
                                stop=(kc == KC1 - 1),
                            )
                        bias = b1_sb[:, e, hm:hm + 1] if b1_nz else 0.0
                        nc.scalar.activation(
                            h_sb[:, hm, c0 + nch:c0 + nch + nn],
                            ps[:, :nn],
                            AF.Relu,
                            bias=bias,
                        )

            # ---- layer 2 + complex-pair normalization, per 128-row block ----
            cur_e = None
            w2t = None
            for mb in range(NBLK):
                e = block_expert[mb]
                if e != cur_e:
                    w2t = w2p.tile([P, KC2, OSL], BF16, tag="w2")
                    nc.sync.dma_start(
                        w2t[:], W2T[e].rearrange("(kc p) n -> p kc n", p=P)
                    )
                    cur_e = e
                ps_a = ps2.tile([P, 512], F32, tag="ps2")
                ps_b = ps2.tile([P, 512], F32, tag="ps2")
                for kc in range(KC2):
                    lhsT = h_sb[:, kc, mb * P:(mb + 1) * P]
                    nc.tensor.matmul(ps_a, lhsT, w2t[:, kc, 0:512],
                                     start=(kc == 0), stop=(kc == KC2 - 1))
                    nc.tensor.matmul(ps_b, lhsT, w2t[:, kc, 512:OSL],
                                     start=(kc == 0), stop=(kc == KC2 - 1))

                halves = [ps_a, ps_b]
                if b2_nz:
                    osb = outp.tile([P, OSL], F32, tag="osb")
                    for i, ph in enumerate(halves):
                        nc.vector.tensor_add(
                            osb[:, i * 512:(i + 1) * 512], ph[:],
                            b2_sb[:, e, i * 512:(i + 1) * 512],
                        )
                    halves = [osb[:, 0:512], osb[:, 512:OSL]]

                sq = sqp.tile([P, OSL], F32, tag="sq")
                for i, ph in enumerate(halves):
                    nc.scalar.activation(sq[:, i * 512:(i + 1) * 512], ph[:],
                                         AF.Square)
                nrm = nrmp.tile([P, OSL // 256], F32, tag="nrm")
                nc.vector.reduce_sum(
                    nrm[:],
                    sq.rearrange("p (w d) -> p w d", d=256),
                    axis=mybir.AxisListType.X,
                )
                nc.scalar.sqrt(nrm[:], nrm[:])
                rn = nrmp.tile([P, OSL // 256], F32, tag="rn")
                nc.vector.reciprocal(rn[:], nrm[:])

                out_sb = outp.tile([P, OSL], F32, tag="onorm")
                for w in range(OSL // 256):
                    src = halves[(w * 256) // 512]
                    off = (w * 256) % 512
                    nc.vector.tensor_scalar_mul(
                        out_sb[:, w * 256:(w + 1) * 256],
                        src[:, off:off + 256],
                        rn[:, w:w + 1],
                    )
                nc.sync.dma_start(OUT[mb * P:(mb + 1) * P, :], out_sb[:])

    nc.compile()
    return nc


def kernel(x, W1, b1, W2, b2):
    x = np.asarray(x, dtype=np.float32)
    W1 = np.asarray(W1, dtype=np.float32)
    b1 = np.asarray(b1, dtype=np.float32)
    W2 = np.asarray(W2, dtype=np.float32)
    b2 = np.asarray(b2, dtype=np.float32)

    route = _route(x)
    pad_idx, valid, segs, block_expert = _plan(route)
    NP = len(pad_idx)

    xt = np.ascontiguousarray(x[pad_idx].T.astype(ml_dtypes.bfloat16))
    w1b = W1.astype(ml_dtypes.bfloat16)
    w2b = W2.astype(ml_dtypes.bfloat16)

    b1_nz = bool(np.any(b1))
    b2_nz = bool(np.any(b2))

    nc = _build_program(NP, segs, block_expert, b1_nz, b2_nz)

    in_maps = []
    for c in range(NCORES):
        sl = slice(c * OSL, (c + 1) * OSL)
        in_maps.append({
            "xt": xt,
            "w1": w1b,
            "b1": b1,
            "w2": np.ascontiguousarray(w2b[:, :, sl]),
            "b2": np.ascontiguousarray(b2[:, sl]),
        })

    trace = os.environ.get("BASSMOE_TRACE", "") == "1"
    if trace:
        trace = _install_ntff_hook()

    res = run_bass_kernel_spmd(
        nc, in_maps, core_ids=list(range(NCORES)), trace=trace,
        tmpdir=os.environ.get("BASSMOE_TRACE_DIR") or None,
    )
    global LAST_EXEC_NS, LAST_TRACE
    LAST_EXEC_NS = res.exec_time_ns
    LAST_TRACE = res.instructions_and_trace[1] if res.instructions_and_trace else None

    out_sorted = np.concatenate(
        [res.results[c]["out"] for c in range(NCORES)], axis=1
    )
    out = np.empty((B, O), dtype=np.float32)
    out[pad_idx[valid]] = out_sorted[valid]
    return out.reshape(B, 32, 256)
